# revision 1
# baseline (speedup 1.0000x reference)
"""Trainium2 Bass kernel for nn_AttitudeController (B=2097152 drones).

Contract: kernel(**inputs) takes the FULL unsharded inputs (numpy) and
returns the FULL [B, 4] float32 output.  Internally the batch is sharded
across 8 NeuronCores; each core runs an identical NEFF on its shard.

Math (derived from the reference):
    R_des^T R = R(q_err),  q_err = q_y(th/2)* x q_x(ph/2)* x q_z(ps/2)* x q
    angle_error = [2ab, 2ac, 0]          (a,b,c,d = q_err components)
    M[:,2]      = [2(bd+ac), 2(cd-ab), 1-2(b^2+c^2)]
    rate_error  = ang_vel - yaw_rate * M[:,2]
    out[r] = sum_k Wf[r,k] * f_k - 1,  f = (2ab, 2ac, re0, re1, re2, thrust)
Wf has +-uniform-magnitude columns for the quad-X mixer, so the final
stage folds into 4 group values G0..G3 and a sign butterfly.

v2 design (engine-balanced):
  - inputs arrive in SBUF as fp16 via SWDGE cast-DMA (f32 HBM -> fp16 SBUF)
  - ACT engine does all strided extractions (q4, eav, u, u2, G3) and the
    sin/cos of the half angles (fp16-strided ACT runs ~1.2ns/elem vs 2.7
    for f32-strided)
  - quaternion q is pre-scaled by sqrt(2*wa) so the P6 products come out
    pre-multiplied by the attitude gain (kills the e13 rescale step)
  - DVE does the quaternion chain + products + tail, all fp16 2x packed
  - outputs are written component-major [P, 4, Cw] (fp16, 2x packed) and
    cast-DMA'd to a transposed [4, SHARD] f32 DRAM tensor; the host
    re-interleaves (cheap numpy fancy-index)
"""

import hashlib
import math

import numpy as np

B_TOTAL = 2097152
N_CORES = 8
SHARD = B_TOTAL // N_CORES          # 262144 rows per core
P = 128                             # SBUF partitions
COLS = SHARD // P                   # 2048 columns per partition

# --- tunables -------------------------------------------------------------
COMPUTE_DT = "float16"              # intermediate dtype on-chip
TILE_WIDTHS = [512, 512, 512, 512]  # column tiling of the 2048 cols
CAST_DMA = True                     # SWDGE f32->fp16 cast on input DMA
MM_OUT = True                       # component-major out (transposed DRAM)
OUT_F32 = True                      # outs write f32, HWDGE out-DMA (frees Pool queue)
IO_BUFS = 3
TMP_BUFS = 2
PARTIAL_OK = False
MAX_WAITS = 1                       # walrus (this build) allows 1 wait/inst

_SQRT2 = float(np.float32(math.sqrt(2.0)))
_PIO2 = float(np.float32(math.pi / 2.0))

# out16 component order is (o0, o3, o1, o2); host maps row k -> column:
OUT_ROW_TO_COL = [0, 3, 1, 2]

_CACHE = {}


# --------------------------------------------------------------------------
# BIR post-processing: this walrus build rejects >1 sync-wait per
# instruction; split offenders into preceding Drain instructions.
# --------------------------------------------------------------------------
_bir_patch_installed = False


def _split_waits_in_bir(bir_bytes):
    import orjson

    d = orjson.loads(bir_bytes)
    changed = False
    mods = d.get("modules", [d]) if "functions" not in d else [d]
    for mod in mods:
        for fn in mod.get("functions", []):
            for blk in fn.get("blocks", []):
                out = []
                for ins in blk.get("instructions", []):
                    si = ins.get("sync_info") or {}
                    waits = si.get("on_wait") or []
                    if len(waits) > MAX_WAITS:
                        changed = True
                        chunks = [
                            waits[i : i + MAX_WAITS]
                            for i in range(0, len(waits), MAX_WAITS)
                        ]
                        for k, ch in enumerate(chunks[:-1]):
                            pre = {
                                "name": f"{ins['name']}-wsplit{k}",
                                "opcode": "Drain",
                                "engine": ins.get("engine", "SP"),
                                "ins": [],
                                "outs": [],
                                "is_reset_sema": False,
                                "sync_info": {"on_update": [], "on_wait": ch},
                            }
                            if "debug" in ins:
                                pre["debug"] = ins["debug"]
                            out.append(pre)
                        si["on_wait"] = chunks[-1]
                        ins["sync_info"] = si
                    out.append(ins)
                blk["instructions"] = out
    if changed:
        return orjson.dumps(d)
    return bir_bytes


def _install_bir_patch():
    global _bir_patch_installed
    if _bir_patch_installed:
        return
    from concourse import bass_utils

    orig = bass_utils.compile_bir_kernel

    def patched(bir_json, tmpdir, neff_name="file.neff", **kw):
        bj = bir_json if isinstance(bir_json, (bytes, bytearray)) else bir_json.encode()
        return orig(_split_waits_in_bir(bytes(bj)), tmpdir, neff_name=neff_name, **kw)

    bass_utils.compile_bir_kernel = patched
    # bass2jax imported the symbol directly
    from concourse import bass2jax

    bass2jax.compile_bir_kernel = patched
    _bir_patch_installed = True


# --------------------------------------------------------------------------
# Parameter folding
# --------------------------------------------------------------------------
def _fold_params(mass, g, mixer, max_thrusts, gain_attitude, gain_angular_rate):
    mixer = np.asarray(mixer, np.float64)
    mt = np.asarray(max_thrusts, np.float64)
    ga = np.asarray(gain_attitude, np.float64)
    gar = np.asarray(gain_angular_rate, np.float64)
    m2 = 2.0 * mixer / mt[:, None]  # [4 rotors, 4]
    Wf = np.zeros((4, 6))
    Wf[:, 0] = -m2[:, 0] * ga[0]     # coeff of 2ab
    Wf[:, 1] = -m2[:, 1] * ga[1]     # coeff of 2ac
    Wf[:, 2] = -m2[:, 0] * gar[0]    # coeff of rate_err0
    Wf[:, 3] = -m2[:, 1] * gar[1]    # coeff of rate_err1
    Wf[:, 4] = -m2[:, 2] * gar[2]    # coeff of rate_err2
    Wf[:, 5] = m2[:, 3] * float(mass) * float(g)

    def col_mag(k):
        m = np.abs(Wf[:, k])
        if not np.allclose(m, m[0], rtol=1e-5):
            raise RuntimeError(f"mixer column {k} magnitudes not uniform: {m}")
        return float(m[0])

    wa, wa1, wr, wr1, wr2, wt = (col_mag(k) for k in range(6))
    if not (np.isclose(wa, wa1, rtol=1e-6) and np.isclose(wr, wr1, rtol=1e-6)):
        raise RuntimeError("asymmetric gains not supported by v2 emitter")
    sA = np.sign(Wf[:, 0]).astype(int)
    sB = np.sign(Wf[:, 1]).astype(int)
    sC = np.sign(Wf[:, 4]).astype(int)
    if not (np.sign(Wf[:, 2]) == sA).all():
        raise RuntimeError("columns 0/2 sign mismatch")
    if not (np.sign(Wf[:, 3]) == sB).all():
        raise RuntimeError("columns 1/3 sign mismatch")
    if not (np.sign(Wf[:, 5]) > 0).all():
        raise RuntimeError("thrust column must be positive")
    return dict(
        wa=wa, wa1=wa1, wr=wr, wr1=wr1, wr2=wr2, wt=wt,
        sA=sA.tolist(), sB=sB.tolist(), sC=sC.tolist(), Wf=Wf,
    )


def folded_numpy(root_state, control_target, fp):
    """Numpy model of exactly what the device computes (fp32). Used by
    test.py to validate the algebra separately from the hardware."""
    q = root_state[:, 3:7].astype(np.float32)
    av = root_state[:, 10:13].astype(np.float32)
    ph = control_target[:, 0]
    th = control_target[:, 1]
    ps = control_target[:, 2]
    t = control_target[:, 3]
    kq = np.float32(math.sqrt(2.0 * fp["wa"]))
    c, s = np.cos(ps / 2), np.sin(ps / 2)
    W, X, Y, Z = (q[:, i] * kq for i in range(4))
    tw = c * W + s * Z
    tx = c * X + s * Y
    ty = c * Y - s * X
    tz = c * Z - s * W
    c, s = np.cos(ph / 2), np.sin(ph / 2)
    uw = c * tw + s * tx
    ux = c * tx - s * tw
    uy = c * ty + s * tz
    uz = c * tz - s * ty
    c, s = np.cos(th / 2), np.sin(th / 2)
    A = c * uw + s * uy
    Bq = c * ux - s * uz
    Cq = c * uy - s * uw
    D = c * uz + s * ux
    AB, AC, BD, CD = A * Bq, A * Cq, Bq * D, Cq * D   # pre-scaled by wa
    M02 = BD + AC
    M12 = CD - AB
    Sg = Bq * Bq + Cq * Cq
    u = ps * np.float32(fp["wr"] / fp["wa"])
    u2s = ps * np.float32(fp["wr2"] / fp["wa"])
    u2b = ps * np.float32(fp["wr2"])
    eav0 = av[:, 0] * np.float32(fp["wr"])
    eav1 = av[:, 1] * np.float32(fp["wr"])
    eav2 = av[:, 2] * np.float32(fp["wr2"])
    G0 = AB + eav0 - u * M02
    G1 = AC + eav1 - u * M12
    G2 = (eav2 - u2b) + u2s * Sg
    G3 = fp["wt"] * t - 1.0
    out = np.empty((root_state.shape[0], 4), np.float32)
    for r in range(4):
        out[:, r] = fp["sA"][r] * G0 + fp["sB"][r] * G1 + fp["sC"][r] * G2 + G3
    return out


# --------------------------------------------------------------------------
# Bass program builder
# --------------------------------------------------------------------------
def _build_nc(fp, reps=1, trace_sim=False):
    import concourse.bass as bass
    import concourse.mybir as mybir
    from concourse.tile import TileContext

    f32 = mybir.dt.float32
    cdt = getattr(mybir.dt, COMPUTE_DT)

    nc = bass.Bass()

    # const APs for the pi/2 bias used by cos-via-sin (both dtypes)
    cbias = nc.alloc_sbuf_tensor("const-f32-pio2", [128, 1], f32)
    nc.gpsimd.memset(cbias.ap(), _PIO2)
    nc.const_aps.aps[(f32, _PIO2)] = cbias.ap()
    cbias16 = nc.alloc_sbuf_tensor("const-f16-pio2", [128, 1], cdt)
    nc.gpsimd.memset(cbias16.ap(), _PIO2)
    nc.const_aps.aps[(cdt, _PIO2)] = cbias16.ap()
    nc.all_engine_barrier()

    rs = nc.declare_dram_parameter("root_state", [SHARD, 13], f32, isOutput=False)
    ct = nc.declare_dram_parameter("control_target", [SHARD, 4], f32, isOutput=False)
    rs2 = rs.rearrange("(p c) m -> p (c m)", p=P)
    ct2 = ct.rearrange("(p c) m -> p (c m)", p=P)
    if MM_OUT:
        out = nc.declare_dram_parameter("out", [4, SHARD], f32, isOutput=True)
        out2 = out.rearrange("m (p c) -> p m c", p=P)
    else:
        out = nc.declare_dram_parameter("out", [SHARD, 4], f32, isOutput=True)
        out2 = out.rearrange("(p c) m -> p (c m)", p=P)

    assert PARTIAL_OK or sum(TILE_WIDTHS) == COLS

    with TileContext(nc, trace_sim=trace_sim) as tc:
        with (
            tc.tile_pool(name="io", bufs=IO_BUFS) as io,
            tc.tile_pool(name="tmp", bufs=TMP_BUFS) as tmp,
        ):
            for rep in range(reps):
                offs = []
                c0 = 0
                for Cw in TILE_WIDTHS:
                    offs.append(c0)
                    c0 += Cw
                nt = len(TILE_WIDTHS)
                ios = {}
                for ti in range(nt):
                    gi = rep * nt + ti
                    if ti == 0:
                        ios[ti] = _emit_in_dma(nc, mybir, io, rs2, ct2,
                                               gi, offs[ti], TILE_WIDTHS[ti], cdt)
                    if ti + 1 < nt:
                        ios[ti + 1] = _emit_in_dma(
                            nc, mybir, io, rs2, ct2,
                            gi + 1, offs[ti + 1], TILE_WIDTHS[ti + 1], cdt)
                    _emit_tile_v2(nc, mybir, io, tmp, ios.pop(ti), out2,
                                  gi, offs[ti], TILE_WIDTHS[ti], fp, cdt,
                                  strided_q=False)
    return nc


def _emit_in_dma(nc, mybir, io, rs2, ct2, ti, c0, Cw, cdt):
    f32 = mybir.dt.float32
    io_dt = cdt if CAST_DMA else f32
    dma_in = nc.gpsimd.dma_start if CAST_DMA else nc.sync.dma_start
    Cmax = max(TILE_WIDTHS)
    # ct first: it is 3.25x smaller and gates the sin/cos extraction that
    # the DVE chain needs first; rs (13 cols) follows on the same queue.
    ct_t = io.tile([P, Cmax * 4], io_dt, tag="ct", name=f"ct_{ti}")[:, : Cw * 4]
    dma_in(out=ct_t, in_=ct2[:, c0 * 4 : (c0 + Cw) * 4])
    rs_t = io.tile([P, Cmax * 13], io_dt, tag="rs", name=f"rs_{ti}")[:, : Cw * 13]
    dma_in(out=rs_t, in_=rs2[:, c0 * 13 : (c0 + Cw) * 13])
    return rs_t, ct_t


def _emit_tile_v2(nc, mybir, io, tmp, io_tiles, out2, ti, c0, Cw, fp, cdt,
                  strided_q=False):
    f32 = mybir.dt.float32
    AF = mybir.ActivationFunctionType
    OP = mybir.AluOpType
    rs_t, ct_t = io_tiles

    rs3 = rs_t.rearrange("p (c m) -> p c m", m=13)
    ct3 = ct_t.rearrange("p (c m) -> p c m", m=4)

    # ---- temp allocator with per-width tag free lists ----
    free_tags = {}
    n_tags = [0]
    tag_of = {}

    Cmax = max(TILE_WIDTHS)

    def alloc(name, k=1):
        fl = free_tags.setdefault(k, [])
        if fl:
            tag = fl.pop()
        else:
            tag = f"w{k}_{n_tags[0]}"
            n_tags[0] += 1
        ap = tmp.tile([P, k * Cmax], cdt, tag=tag, name=f"{name}_{ti}")[:, : k * Cw]
        tag_of[id(ap)] = (tag, k)
        return ap

    def freet(*aps):
        for ap in aps:
            tag, k = tag_of.pop(id(ap))
            free_tags[k].append(tag)

    def v(ap, k):
        return ap.rearrange("p (k c) -> p k c", c=Cw)

    def bc(ap_pc, k):
        """broadcast a [P, Cw] AP across k components -> [P, k, Cw]"""
        return (ap_pc.rearrange("p (k c) -> p k c", k=1)
                .to_broadcast([P, k, Cw]))

    def bc4d(ap_pc):
        return (ap_pc.rearrange("p (a b c) -> p a b c", a=1, b=1)
                .to_broadcast([P, 2, 2, Cw]))

    TT = nc.vector.tensor_tensor

    def act(dst, in_ap, func=AF.Copy, scale=1.0, bias=0.0):
        nc.scalar.activation(dst, in_ap, func, bias=bias, scale=scale)

    kq = math.sqrt(2.0 * fp["wa"])

    # =========== ACT: extractions + trig ===========
    # sin/cos of half angles, batched over the 3 angle columns.
    # cs6 = [P, 6, Cw]: comps 0..2 = cos(a_j/2), comps 3..5 = sin(a_j/2)
    cs6 = alloc("cs6", 6)
    cs6v = v(cs6, 6)
    ang_src = (ct3[:, :, 0:3].rearrange("p c m -> p m c"))  # [P, 3, Cw]
    act(cs6v[:, 0:3], ang_src, AF.Sin, scale=0.5, bias=_PIO2)
    act(cs6v[:, 3:6], ang_src, AF.Sin, scale=0.5)
    # q4 = sqrt(2*wa) * (w, x, y, z)  -> [P, 4, Cw]
    q4 = alloc("q4", 4)
    q4v = v(q4, 4)
    for i, j in enumerate((3, 4, 5, 6)):
        act(q4v[:, i], rs3[:, :, j], scale=kq)
    # eav = (wr*av0, wr*av1, wr2*av2) -> [P, 3, Cw]
    eav = alloc("eav", 3)
    eavv = v(eav, 3)
    act(eavv[:, 0], rs3[:, :, 10], scale=fp["wr"])
    act(eavv[:, 1], rs3[:, :, 11], scale=fp["wr"])
    act(eavv[:, 2], rs3[:, :, 12], scale=fp["wr2"])
    # u-scalars: u from ACT; u2s/u2b derived on DVE (tensor_scalar, 4x)
    u = alloc("u")
    act(u[:], ct3[:, :, 2], scale=fp["wr"] / fp["wa"])
    u2s = alloc("u2s")
    nc.vector.tensor_scalar(u2s[:], u[:], fp["wr2"] / fp["wr"], None, OP.mult)
    u2b = alloc("u2b")
    nc.vector.tensor_scalar(u2b[:], u[:], fp["wr2"] * fp["wa"] / fp["wr"], None,
                            OP.mult)

    def cosc(j):
        return cs6v[:, j]

    def sinc(j):
        return cs6v[:, 3 + j]

    # =========== DVE: quaternion chain ===========
    # stage 1: q_z* x q   (pairs (W,Z),(X,Y) rotated by psi/2) -> angle 2
    mc = alloc("mc", 4); ms = alloc("ms", 4)
    mcv = v(mc, 4); msv = v(ms, 4)
    TT(mcv[:, :], bc(cosc(2), 4), q4v[:, :], OP.mult)
    TT(msv[:, :], bc(sinc(2), 4), q4v[:, ::-1], OP.mult)
    t4 = alloc("t4", 4)
    t4v = v(t4, 4)
    TT(t4v[:, 0:2], mcv[:, 0:2], msv[:, 0:2], OP.add)
    TT(t4v[:, 2:4], mcv[:, 2:4], msv[:, 2:4], OP.subtract)
    freet(q4)

    # stage 2: q_x* x t  (swap within pairs) -> angle 0 (roll)
    TT(mcv[:, :], bc(cosc(0), 4), t4v[:, :], OP.mult)
    ms4d = ms.rearrange("p (a b c) -> p a b c", a=2, c=Cw)
    t4sw = t4.rearrange("p (a b c) -> p a b c", a=2, c=Cw)[:, :, ::-1]
    TT(ms4d, bc4d(sinc(0)), t4sw, OP.mult)
    u4 = alloc("u4", 4)
    u4v = v(u4, 4)
    TT(u4v[:, 0:4:2], mcv[:, 0:4:2], msv[:, 0:4:2], OP.add)
    TT(u4v[:, 1:4:2], mcv[:, 1:4:2], msv[:, 1:4:2], OP.subtract)
    freet(t4)

    # stage 3: q_y* x u  (rotate-2) -> angle 1 (pitch)
    TT(mcv[:, :], bc(cosc(1), 4), u4v[:, :], OP.mult)
    ms4r = ms.rearrange("p (a b c) -> p a b c", b=2, c=Cw)
    u4rot = u4.rearrange("p (a b c) -> p a b c", b=2, c=Cw)[:, ::-1]
    TT(ms4r, bc4d(sinc(1)), u4rot, OP.mult)
    a4 = alloc("a4", 4)
    a4v = v(a4, 4)
    TT(a4v[:, 0:4:3], mcv[:, 0:4:3], msv[:, 0:4:3], OP.add)
    TT(a4v[:, 1:3], mcv[:, 1:3], msv[:, 1:3], OP.subtract)
    freet(u4, mc, ms, cs6)

    # =========== DVE: products (pre-scaled by wa) ===========
    # P6 = (AB, AC, BD, CD, BB, CC)
    P6 = alloc("P6", 6)
    P6v = v(P6, 6)
    TT(P6v[:, 0:2], bc(a4v[:, 0], 2), a4v[:, 1:3], OP.mult)
    TT(P6v[:, 2:4], a4v[:, 1:3], bc(a4v[:, 3], 2), OP.mult)
    TT(P6v[:, 4:6], a4v[:, 1:3], a4v[:, 1:3], OP.mult)
    freet(a4)

    # GB = (G3, G1, G0, G2); G3 written by ACT (reuses a freed 4-wide tag)
    GB = alloc("GB", 4)
    GBv = v(GB, 4)
    act(GBv[:, 0], ct3[:, :, 3], scale=fp["wt"], bias=-1.0)

    # M3 = (M02, Sg, M12)
    M3 = alloc("M3", 3)
    M3v = v(M3, 3)
    # (M02, Sg) = (BD, BB) + (AC, CC)
    TT(M3v[:, 0:2], P6v[:, 2:6:2], P6v[:, 1:6:4], OP.add)
    TT(M3v[:, 2], P6v[:, 3], P6v[:, 0], OP.subtract)

    # t01 = (AB, AC) + (eav0, eav1)
    t01 = alloc("t01", 2)
    TT(v(t01, 2)[:, :], P6v[:, 0:2], eavv[:, 0:2], OP.add)
    freet(P6)

    # s01 = u * (M02, M12)
    s01 = alloc("s01", 2)
    TT(v(s01, 2)[:, :], bc(u[:], 2), M3v[:, 0:3:2], OP.mult)
    # s2 = u2s * Sg
    s2 = alloc("s2")
    TT(s2[:], u2s[:], M3v[:, 1], OP.mult)
    freet(M3, u, u2s)

    # (G0, G1) -> GB comps (2, 1)
    TT(GBv[:, 2:0:-1], v(t01, 2)[:, :], v(s01, 2)[:, :], OP.subtract)
    # G2 = (eav2 - u2b) + s2
    g2a = alloc("g2a")
    TT(g2a[:], eavv[:, 2], u2b[:], OP.subtract)
    TT(GBv[:, 3], g2a[:], s2[:], OP.add)
    freet(t01, s01, s2, g2a, u2b, eav)

    # =========== butterfly + outs ===========
    # UVt = (U-, U+, V+, V-);  U+- = G3 +- G0, V+- = G1 +- G2
    UVt = alloc("UV", 4)
    UVv = v(UVt, 4)
    TT(UVv[:, 0:4:3], GBv[:, 0:2], GBv[:, 2:4], OP.subtract)
    TT(UVv[:, 1:3], GBv[:, 0:2], GBv[:, 2:4], OP.add)
    freet(GB)

    # out rows (o0, o3, o1, o2):
    #   (o0, o3) = (U-, U+) + (V+, V-);  (o1, o2) = (U-, U+) - (V+, V-)
    # validity for generic sign patterns is asserted in kernel()
    if MM_OUT:
        odt = f32 if OUT_F32 else cdt
        out_t = io.tile([P, max(TILE_WIDTHS) * 4], odt, tag="out",
                        name=f"out_{ti}")[:, : Cw * 4]
        ov = out_t.rearrange("p (k c) -> p k c", c=Cw)
        TT(ov[:, 0:2], UVv[:, 0:2], UVv[:, 2:4], OP.add)
        TT(ov[:, 2:4], UVv[:, 0:2], UVv[:, 2:4], OP.subtract)
        freet(UVt)
        dma_out = nc.sync.dma_start if OUT_F32 else nc.gpsimd.dma_start
        dma_out(out=out2[:, :, c0 : c0 + Cw], in_=ov[:, :, :])
    else:
        out_t = io.tile([P, Cw * 4], f32, tag="out", name=f"out_{ti}")
        out3 = out_t.rearrange("p (c m) -> p c m", m=4)
        uidx = {1: 1, -1: 0}
        vidx = {1: 2, -1: 3}
        for r in range(4):
            uu = UVv[:, uidx[fp["sA"][r]]]
            vv = UVv[:, vidx[fp["sB"][r] * fp["sC"][r]]]
            op = OP.add if fp["sB"][r] > 0 else OP.subtract
            TT(out3[:, :, r], uu, vv, op)
        freet(UVt)
        nc.sync.dma_start(out=out2[:, c0 * 4 : (c0 + Cw) * 4], in_=out_t[:])


# --------------------------------------------------------------------------
# Public entry point
# --------------------------------------------------------------------------
def kernel(root_state, control_target, mass, g, mixer, max_thrusts,
           gain_attitude, gain_angular_rate):
    root_state = np.ascontiguousarray(np.asarray(root_state, np.float32))
    control_target = np.ascontiguousarray(np.asarray(control_target, np.float32))
    assert root_state.shape == (B_TOTAL, 13), root_state.shape
    assert control_target.shape == (B_TOTAL, 4), control_target.shape

    fp = _fold_params(mass, g, mixer, max_thrusts, gain_attitude, gain_angular_rate)
    if MM_OUT:
        # the m-major butterfly hardcodes the quad-X sign pattern:
        assert fp["sA"] == [-1, -1, 1, 1], fp["sA"]
        assert fp["sB"] == [1, -1, -1, 1], fp["sB"]
        assert fp["sC"] == [1, -1, 1, -1], fp["sC"]

    key = hashlib.sha256(
        repr(({k: v for k, v in fp.items() if k != "Wf"}, COMPUTE_DT,
              tuple(TILE_WIDTHS), CAST_DMA, MM_OUT, OUT_F32, IO_BUFS,
              TMP_BUFS)).encode()
    ).hexdigest()
    if key not in _CACHE:
        _install_bir_patch()
        _CACHE[key] = _build_nc(fp)
    nc = _CACHE[key]

    from concourse.bass_utils import run_bass_kernel_spmd

    rs_shards = root_state.reshape(N_CORES, SHARD, 13)
    ct_shards = control_target.reshape(N_CORES, SHARD, 4)
    in_maps = [
        {"root_state": rs_shards[i], "control_target": ct_shards[i]}
        for i in range(N_CORES)
    ]
    res = run_bass_kernel_spmd(nc, in_maps, core_ids=list(range(N_CORES)))
    return gather_out(res)


def gather_out(res, n_cores=N_CORES):
    if MM_OUT:
        outs = np.stack([res.results[i]["out"] for i in range(n_cores)])  # [n,4,SHARD]
        full = np.empty((n_cores * SHARD, 4), np.float32)
        fullv = full.reshape(n_cores, SHARD, 4)
        for k, col in enumerate(OUT_ROW_TO_COL):
            fullv[:, :, col] = outs[:, k, :]
        return full
    return np.concatenate([res.results[i]["out"] for i in range(n_cores)], axis=0)

